# revision 6
# baseline (speedup 1.0000x reference)
"""Trainium2 Bass kernel for segmented per-(d,k) 1D conv (PartiallyUnsharedConv1d).

Problem (hardcoded):
  x      [B=4, D=32, K=8, CI=2, L=4096] f32
  weight [D, K, CO=2, CI, S=8, 1, NB=15] f32
  bias   [D, K, CO, S, 1] f32
  out    [B, D, K, CO, L] f32

  out[b,d,k,o,l] = sum_{i,f} weight[d,k,o,i,seg(l),0,f] * xpad[b,d,k,i,l+f]
                   + bias[d,k,o,seg(l),0]
  where xpad is x zero-padded by P=7 on both ends of l, seg(l) assigns l to one
  of 8 contiguous segments (7x499 + 603).

Sharding: 8 cores = 4 d-groups x 2 b-groups. Each core owns 64 (d,k) pairs and
2 batch entries. Per core all 128 SBUF partitions are filled with (dk, i) rows;
a block-diagonal (64 blocks of 2x2) stationary matrix per (segment, tap) turns
the whole per-core conv into 15 PSUM-accumulated matmuls per output tile, with
the tap shift realized as a shifted SBUF slice of the padded x. No cross-core
communication.

Everything on-chip runs in bf16 (x, weights, bias, output) with fp32 PSUM
accumulation: the PE streams bf16 at the same 1 column/cycle as fp32r, but all
DMA traffic halves; output rel-err ~2.3e-3, well under the 2e-2 gate.
"""

import numpy as np

# problem dims
B, D, K, CI, CO, L, NB, P, S = 4, 32, 8, 2, 2, 4096, 15, 7, 8
LP = L + 2 * P  # 4110
LX = 4112  # bf16 row length (16B-aligned rows; max read col is 4110)

# segment layout (replicates reference _segment_ids)
_rough = LP // S
SEG_LENS = [_rough - 2 * P] * (S - 1)  # 499 x 7
SEG_LENS.append(L - sum(SEG_LENS))  # 603
SEG_STARTS = np.concatenate([[0], np.cumsum(SEG_LENS)[:-1]]).tolist()

# sharding
N_CORES = 8
DG, BG = 4, 2  # d-groups x b-groups
D_PER = D // DG  # 8
B_PER = B // BG  # 2
DK = D_PER * K  # 64 (d,k) pairs per core
NPART = 128
MAX_N = 512  # fp32 PSUM bank limit

_prog_cache = {}


def _subtiles(s):
    """(t0, n) output tiles for segment s (PSUM free-dim <= 512)."""
    start, ln = SEG_STARTS[s], SEG_LENS[s]
    if ln <= MAX_N:
        return [(start, ln)]
    h = ln // 2
    return [(start, h), (start + h, ln - h)]


# meta tensor per-partition layout (bf16 elements):
#   [0:128)            block-diag mask: mask[p, m] = (p//2 == m//2)
#   [128:144)          bias as f32 bit-packed into bf16 pairs, f32 col = s
#   [144:144+8*30)     compact weights, seg s block at 144+30s,
#                      within block col = f*CO + o, row = (dk, i)
OFF_MASK = 0
OFF_BIAS = NPART  # 128
OFF_W = OFF_BIAS + 2 * S  # 144
SEG_W = NB * CO  # 30
TOT_META = OFF_W + S * SEG_W  # 384
META_A = OFF_W + SEG_W  # first meta chunk: mask + bias + seg0 weights


def _build_program(compute_dt="bfloat16", loop_n=None, full_loop=False):
    import contextlib

    import concourse.mybir as mybir
    import concourse.tile as tile
    from concourse import bacc

    cdt = getattr(mybir.dt, compute_dt)
    f32 = mybir.dt.float32

    nc = bacc.Bacc("TRN2", target_bir_lowering=False, debug=False)

    meta_d = nc.dram_tensor("meta", [NPART, TOT_META], cdt, kind="ExternalInput").ap()
    xa_d = nc.dram_tensor("xa", [NPART, LX], cdt, kind="ExternalInput").ap()
    xb_d = nc.dram_tensor("xb", [NPART, LX], cdt, kind="ExternalInput").ap()
    out_d = nc.dram_tensor("out", [NPART, B_PER, L], cdt, kind="ExternalOutput").ap()

    with tile.TileContext(nc) as tc:
        with (
            tc.tile_pool(name="const", bufs=1) as cpool,
            tc.tile_pool(name="psum", bufs=8, space="PSUM") as ppool,
        ):
            meta = cpool.tile([NPART, TOT_META], cdt, tag="meta", name="meta")
            x_tiles = [
                cpool.tile([NPART, LX], cdt, tag=f"x{b}", name=f"x{b}")
                for b in range(B_PER)
            ]
            w_tiles = [
                cpool.tile([NPART, NB * NPART], cdt, tag=f"w{s}", name=f"w{s}")
                for s in range(S)
            ]
            out_t = cpool.tile([NPART, B_PER, L], cdt, tag="out", name="out")

            mask_2d = meta[:, OFF_MASK : OFF_MASK + NPART].rearrange(
                "p (m o) -> p m o", o=CO
            )
            mask_3d = (
                meta[:, OFF_MASK : OFF_MASK + NPART]
                .rearrange("p (u m) -> p u m", u=1)
                .broadcast_to((NPART, NB, NPART))
            )

            def seg_w(s):
                # [p, NB, 1, CO] compact weight block for segment s
                return meta[:, OFF_W + SEG_W * s : OFF_W + SEG_W * (s + 1)].rearrange(
                    "p (f u o) -> p f u o", f=NB, u=1
                )

            def bias_sl(s):
                return meta[:, OFF_BIAS + 2 * s : OFF_BIAS + 2 * s + 2].bitcast(f32)

            def emit_input_dma():
                # meta chunk A (mask+bias+seg0 weights) and x0 chunk 0 run
                # concurrently — both gate the first matmul (~0.5us together).
                # Everything later is chained so the early chunks get full
                # bandwidth.
                dma_a = nc.sync.dma_start(out=meta[:, :META_A], in_=meta_d[:, :META_A])
                chain = [dma_a]
                XCUTS = (0, 528, 2016, LX)  # seg0 | segs1-3 | rest
                first = True
                for b in range(B_PER):
                    xd = xa_d if b == 0 else xb_d
                    for lo, hi in zip(XCUTS, XCUTS[1:]):
                        chain.append(
                            nc.sync.dma_start(
                                out=x_tiles[b][:, lo:hi], in_=xd[:, lo:hi]
                            )
                        )
                        if first:
                            # meta chunk B right after the two gating chunks
                            chain.append(
                                nc.sync.dma_start(
                                    out=meta[:, META_A:], in_=meta_d[:, META_A:]
                                )
                            )
                            first = False
                for prev, nxt in zip(chain[1:], chain[2:]):
                    tile.add_dep_helper(
                        nxt.ins, prev.ins, sync=True, reason="serialize input DMA"
                    )

            def emit_weight_build():
                # Pool engine builds all stationary tiles (DVE is reserved for
                # the PSUM->out bias stage). Segment 0 is built per-tap so tap
                # 0 lands ~150ns after meta chunk A and the PE can start.
                for s in range(S):
                    if s == 0:
                        for f in range(NB):
                            base = OFF_W + f * CO
                            nc.gpsimd.tensor_mul(
                                w_tiles[0][:, f * NPART : (f + 1) * NPART].rearrange(
                                    "p (m o) -> p m o", o=CO
                                ),
                                meta[:, base : base + CO]
                                .rearrange("p (u o) -> p u o", u=1)
                                .broadcast_to((NPART, DK, CO)),
                                mask_2d,
                            )
                    else:
                        nc.gpsimd.tensor_mul(
                            w_tiles[s][:, :].rearrange("p (f m) -> p f m", m=NPART),
                            seg_w(s).broadcast_to((NPART, NB, DK, CO)),
                            mask_3d,
                        )

            def emit_body():
                for b in range(B_PER):
                    for s in range(S):
                        for (t0, n) in _subtiles(s):
                            ps = ppool.tile([NPART, MAX_N], f32, tag="ps", name="ps")
                            for f in range(NB):
                                nc.tensor.matmul(
                                    ps[:, :n],
                                    lhsT=w_tiles[s][:, f * NPART : (f + 1) * NPART],
                                    rhs=x_tiles[b][:, t0 + f : t0 + f + n],
                                    start=(f == 0),
                                    stop=(f == NB - 1),
                                )
                            nc.vector.tensor_scalar_add(
                                out_t[:, b, t0 : t0 + n], ps[:, :n], bias_sl(s)
                            )

            def emit_output_dma():
                # Per-(b, segment) drains: each goes as soon as its bias-add
                # lands; only the last ~0.15 MB trails the final compute.
                for b in range(B_PER):
                    for s in range(S):
                        t0, ln = SEG_STARTS[s], SEG_LENS[s]
                        nc.sync.dma_start(
                            out=out_d[:, b, t0 : t0 + ln],
                            in_=out_t[:, b, t0 : t0 + ln],
                        )

            if loop_n is not None:
                loop_ctx = tc.For_i(
                    0,
                    loop_n,
                    1,
                    hint_engines=(mybir.EngineType.PE,),
                    staggered_reset=True,
                )
            else:
                loop_ctx = contextlib.nullcontext()

            if full_loop and loop_n is not None:
                with loop_ctx:
                    emit_input_dma()
                    emit_weight_build()
                    emit_body()
                    emit_output_dma()
            else:
                emit_input_dma()
                emit_weight_build()
                with loop_ctx:
                    emit_body()
                emit_output_dma()

    nc.compile()
    return nc


def _np_dtype_for(compute_dt):
    if compute_dt == "bfloat16":
        import ml_dtypes

        return ml_dtypes.bfloat16
    if compute_dt == "float16":
        return np.float16
    return np.float32


def _shard_inputs(x, w, bias, compute_dt="bfloat16"):
    """Host-side reshape into per-core DRAM layouts."""
    ndt = _np_dtype_for(compute_dt)
    xp = np.pad(x, [(0, 0)] * 4 + [(P, P)])  # [B,D,K,CI,LP]
    in_maps = []
    for core in range(N_CORES):
        dg, bg = divmod(core, BG)
        dsl = slice(dg * D_PER, (dg + 1) * D_PER)
        bsl = slice(bg * B_PER, (bg + 1) * B_PER)

        # x: partitions (d,k,i), cols l, rows zero-extended LP -> LX
        xs = xp[bsl, dsl]
        x_core = np.zeros((B_PER, NPART, LX), np.float32)
        x_core[:, :, :LP] = xs.transpose(0, 1, 2, 3, 4).reshape(
            B_PER, D_PER * K * CI, LP
        )
        # note: xs is [B_PER, D_PER, K, CI, LP] -> partitions (d,k,i)
        # reshape above keeps (d,k,i) order per b

        # compact weights: per segment block, col (f, o), row (dk, i)
        wd = w[dsl, :, :, :, :, 0, :].reshape(DK, CO, CI, S, NB)
        # wcomp[s][p=(dk,i), f*CO+o] = w[dk, o, i, s, f]
        wcomp = np.ascontiguousarray(
            wd.transpose(3, 0, 2, 4, 1).reshape(S, NPART, NB * CO)
        )

        # block-diag mask
        p = np.arange(NPART)
        mask = (p[:, None] // CO == p[None, :] // CO).astype(np.float32)

        # bias: row (dk, o), col s — f32 bits packed as bf16 pairs
        bias_core = np.ascontiguousarray(
            bias[dsl, :, :, :, 0].reshape(NPART, S).astype(np.float32)
        )
        import ml_dtypes

        bias_packed = bias_core.view(ml_dtypes.bfloat16).astype(np.float32)

        meta = np.concatenate(
            [mask, bias_packed] + [wcomp[s] for s in range(S)], axis=1
        )
        meta_nd = np.ascontiguousarray(meta).astype(ndt)
        # re-stamp the exact f32 bias bits (the astype above round-trips them)
        meta_nd[:, OFF_BIAS : OFF_BIAS + 2 * S] = bias_core.view(ml_dtypes.bfloat16)
        in_maps.append(
            {
                "meta": meta_nd,
                "xa": np.ascontiguousarray(x_core[0]).astype(ndt),
                "xb": np.ascontiguousarray(x_core[1]).astype(ndt),
            }
        )
    return in_maps


def _unshard_output(results):
    out = np.empty((B, D, K, CO, L), np.float32)
    for core in range(N_CORES):
        dg, bg = divmod(core, BG)
        oc = results[core]["out"].astype(np.float32).reshape(D_PER, K, CO, B_PER, L)
        out[bg * B_PER : (bg + 1) * B_PER, dg * D_PER : (dg + 1) * D_PER] = (
            oc.transpose(3, 0, 1, 2, 4)
        )
    return out


def run(inputs, trace=False, compute_dt="bfloat16"):
    """Returns (output ndarray, BassKernelResults)."""
    from concourse.bass_utils import run_bass_kernel_spmd

    x = np.asarray(inputs["x"], np.float32)
    w = np.asarray(inputs["weight"], np.float32)
    bias = np.asarray(inputs["bias"], np.float32)

    key = (compute_dt,)
    if key not in _prog_cache:
        _prog_cache[key] = _build_program(compute_dt)
    nc = _prog_cache[key]

    in_maps = _shard_inputs(x, w, bias, compute_dt)
    res = run_bass_kernel_spmd(nc, in_maps, list(range(N_CORES)), trace=trace)
    return _unshard_output(res.results), res


def kernel(**inputs) -> np.ndarray:
    out, _ = run(inputs)
    return out


def _make_callable(nc):
    """One-time jitted shard_map callable for a bass program; zeros for the
    output operands are generated inside the jit (no donation needed)."""
    import jax
    import jax.numpy as jnp
    from jax.experimental.shard_map import shard_map
    from jax.sharding import Mesh, PartitionSpec

    import concourse.mybir as mybir
    from concourse import bass2jax

    bass2jax.install_neuronx_cc_hook()

    partition_name = nc.partition_id_tensor.name if nc.partition_id_tensor else None
    in_names, out_names, out_avals = [], [], []
    for alloc in nc.m.functions[0].allocations:
        if not isinstance(alloc, mybir.MemoryLocationSet):
            continue
        name = alloc.memorylocations[0].name
        if alloc.kind == "ExternalInput":
            if name != partition_name:
                in_names.append(name)
        elif alloc.kind == "ExternalOutput":
            out_names.append(name)
            out_avals.append(
                jax.core.ShapedArray(tuple(alloc.tensor_shape), mybir.dt.np(alloc.dtype))
            )
    n_params = len(in_names)
    all_names = in_names + out_names + ([partition_name] if partition_name else [])

    def _body(*args):
        operands = list(args)
        if partition_name is not None:
            operands.append(bass2jax.partition_id_tensor())
        return tuple(
            bass2jax._bass_exec_p.bind(
                *operands,
                out_avals=tuple(out_avals),
                in_names=tuple(all_names),
                out_names=tuple(out_names),
                lowering_input_output_aliases=(),
                sim_require_finite=True,
                sim_require_nnan=True,
                nc=nc,
            )
        )

    n_outs = len(out_names)
    devices = jax.devices()[:N_CORES]
    mesh = Mesh(np.asarray(devices), ("core",))
    sharding = jax.sharding.NamedSharding(mesh, PartitionSpec("core"))
    jitted = jax.jit(
        shard_map(
            _body,
            mesh=mesh,
            in_specs=(PartitionSpec("core"),) * (n_params + n_outs),
            out_specs=(PartitionSpec("core"),) * n_outs,
            check_rep=False,
        ),
        donate_argnums=tuple(range(n_params, n_params + n_outs)),
        keep_unused=True,
    )

    def _zeros():
        return [
            jax.device_put(
                np.zeros((N_CORES * av.shape[0], *av.shape[1:]), av.dtype), sharding
            )
            for av in out_avals
        ]

    return jitted, in_names, _zeros, sharding


def bench(inputs, compute_dt="bfloat16", n_lo=16, n_hi=616, iters=7, full_loop=True):
    """Per-iteration HW time from the slope between two hardware-loop trip
    counts inside single NEFF executions (the ~100 ms axon dispatch floor
    cancels out).  full_loop=True wraps DMA+build+body+drain per iteration —
    a proxy for the graded single-shot span."""
    import time

    import jax

    x = np.asarray(inputs["x"], np.float32)
    w = np.asarray(inputs["weight"], np.float32)
    bias = np.asarray(inputs["bias"], np.float32)
    in_maps = _shard_inputs(x, w, bias, compute_dt)

    calls = {}
    concat_in = None
    for n in (n_lo, n_hi):
        key = (compute_dt, "loop", n, full_loop)
        if key not in _prog_cache:
            _prog_cache[key] = _build_program(compute_dt, loop_n=n, full_loop=full_loop)
        jitted, in_names, zeros_fn, sharding = _make_callable(_prog_cache[key])
        if concat_in is None:
            concat_in = [
                jax.device_put(
                    np.concatenate([in_maps[c][nm] for c in range(N_CORES)], axis=0),
                    sharding,
                )
                for nm in in_names
            ]
        calls[n] = (jitted, zeros_fn)

    for n in (n_lo, n_hi):
        jitted, zeros_fn = calls[n]
        jax.block_until_ready(jitted(*concat_in, *zeros_fn()))
        time.sleep(0.2)
    diffs = []
    for _ in range(iters):
        pair = {}
        for n in (n_lo, n_hi):
            jitted, zeros_fn = calls[n]
            z = zeros_fn()
            jax.block_until_ready(z)
            t0 = time.perf_counter()
            jax.block_until_ready(jitted(*concat_in, *z))
            pair[n] = time.perf_counter() - t0
            time.sleep(0.1)
        diffs.append(pair[n_hi] - pair[n_lo])
        print(
            f"  pair: lo {pair[n_lo] * 1e3:.2f} ms  hi {pair[n_hi] * 1e3:.2f} ms"
            f"  diff {(pair[n_hi] - pair[n_lo]) * 1e3:.2f} ms"
        )
    diffs.sort()
    med = diffs[len(diffs) // 2]
    slope_ns = med / (n_hi - n_lo) * 1e9
    print(f"  per-iteration time: {slope_ns:.0f} ns")
    return slope_ns
